# revision 21
# baseline (speedup 1.0000x reference)
"""Causal self-attention (B=2, T=2048, D=2048, H=16, HD=128) on 8 trn2 cores.

Sharding: tensor-parallel over heads (2 heads/core). Each core computes
q/k/v projections for its heads from the full input, runs attention, and
produces a partial output projection (its heads' slice of wo's contraction);
the host sums the 8 partials. k/v caches are written directly per head-slice.

Layout choices:
  - x is pre-transposed on host to xT [D, B*T] so projections produce
    qT/kT [HD, T] (head dim on partitions) with no on-chip transposes.
  - q/k head-dims are permuted (even indices first, odd second) via a host
    permutation of the wq/wk rows, so RoPE's even/odd pair-mixing becomes
    partition-half arithmetic on the DVE. The permutation cancels in q.kT.
  - scores are computed transposed (scoresT [tk, tq]) so the exp'd
    probabilities feed the PV matmul directly as the moving operand.
    Softmax is max-free (inputs are well-scaled); the denominator comes from
    an ones-vector matmul accumulated alongside PV, and is applied to
    attnvT columns via reciprocal + gpsimd partition-broadcast + DVE mult.
  - The attention output attnvT [d, tq] is exactly the lhsT the wo
    projection needs; its output goes PSUM -> DRAM directly.
"""

import math

import ml_dtypes
import numpy as np

import concourse.bacc as bacc
import concourse.mybir as mybir
import concourse.tile as tile
from concourse.bass_utils import run_bass_kernel_spmd

B, T, D, H, HD = 2, 2048, 2048, 16, 128
NCORES = 8
HPC = H // NCORES          # heads per core
DL = HPC * HD              # local head-dim span (256)
BT = B * T
KT = D // 128              # contraction tiles for projections
TQ = 512                   # query block (free dim of scoresT)
NTQ = T // TQ
TK = 128                   # key block (partition dim of scoresT)
NTK = T // TK
SCALE = 1.0 / math.sqrt(HD)
BF16 = mybir.dt.bfloat16
F32 = mybir.dt.float32

_CACHE = {}
LAST_RESULTS = None


def _build_program(mode):
    """mode: 'causal' (skip masked blocks, binary-mask diagonal),
    'zeros' (no mask at all), 'general' (additive mask streamed from DRAM)."""
    nc = bacc.Bacc("TRN2", target_bir_lowering=False, debug=False,
                   num_devices=NCORES)

    xT_d = nc.dram_tensor("xT", (D, BT), BF16, kind="ExternalInput").ap()
    wqT_d = nc.dram_tensor("wqT", (D, DL), BF16, kind="ExternalInput").ap()
    wkT_d = nc.dram_tensor("wkT", (D, DL), BF16, kind="ExternalInput").ap()
    wvT_d = nc.dram_tensor("wvT", (D, DL), BF16, kind="ExternalInput").ap()
    woT_d = nc.dram_tensor("woT", (DL, D), BF16, kind="ExternalInput").ap()
    cosT_d = nc.dram_tensor("cosT", (128, T), F32, kind="ExternalInput").ap()
    pmat_d = nc.dram_tensor("pmat", (128, 128), F32, kind="ExternalInput").ap()
    sinT2_d = nc.dram_tensor("sinT2", (128, T), F32, kind="ExternalInput").ap()
    if mode == "causal":
        bmask_d = nc.dram_tensor("bmask", (4, 128, TQ), BF16,
                                 kind="ExternalInput").ap()
    if mode == "general":
        maskTs_d = nc.dram_tensor("maskTs", (T, T), F32,
                                  kind="ExternalInput").ap()

    out_d = nc.dram_tensor("part", (BT, D), BF16, kind="ExternalOutput").ap()
    k_d = nc.dram_tensor("kc", (B, T, HPC, HD), F32, kind="ExternalOutput").ap()
    v_d = nc.dram_tensor("vc", (B, T, HPC, HD), F32, kind="ExternalOutput").ap()

    with tile.TileContext(nc) as tc:
        with (
            tc.tile_pool(name="const", bufs=1) as cpool,
            tc.tile_pool(name="resid", bufs=1) as rpool,
            tc.tile_pool(name="xt", bufs=2) as xpool,
            tc.tile_pool(name="rope", bufs=4) as tpool,
            tc.tile_pool(name="kf", bufs=3) as kfpool,
            tc.tile_pool(name="mk", bufs=3) as mkpool,
            tc.tile_pool(name="et", bufs=6) as etpool,
            tc.tile_pool(name="row", bufs=2) as rowpool,
            tc.tile_pool(name="bcast", bufs=2) as bcpool,
            tc.tile_pool(name="kvs", bufs=3) as kvspool,
            tc.tile_pool(name="os", bufs=3) as ospool,
            tc.tile_pool(name="ost", bufs=2) as ostpool,
            tc.tile_pool(name="ps_s", bufs=4, space="PSUM") as ps_s,
            tc.tile_pool(name="ps_sm", bufs=2, space="PSUM") as ps_sm,
            tc.tile_pool(name="ps_pv", bufs=2, space="PSUM") as ps_pv,
        ):
            # ---- constants ----
            wqt = cpool.tile([128, KT * DL], BF16)
            wkt = cpool.tile([128, KT * DL], BF16)
            wvt = cpool.tile([128, KT * DL], BF16)
            wot = cpool.tile([128, 2 * D], BF16)
            cost = cpool.tile([128, T], F32)
            sint = cpool.tile([128, T], F32)
            ones = cpool.tile([128, 1], BF16)
            pmat = cpool.tile([128, 128], F32)
            nc.sync.dma_start(out=wqt.rearrange("p (k m) -> p k m", k=KT),
                              in_=wqT_d.rearrange("(k p) m -> p k m", p=128))
            nc.sync.dma_start(out=wkt.rearrange("p (k m) -> p k m", k=KT),
                              in_=wkT_d.rearrange("(k p) m -> p k m", p=128))
            nc.sync.dma_start(out=wvt.rearrange("p (k m) -> p k m", k=KT),
                              in_=wvT_d.rearrange("(k p) m -> p k m", p=128))
            nc.sync.dma_start(out=wot.rearrange("p (k m) -> p k m", k=2),
                              in_=woT_d.rearrange("(k p) m -> p k m", p=128))
            nc.sync.dma_start(out=cost[:, :], in_=cosT_d)
            nc.sync.dma_start(out=sint[:, :], in_=sinT2_d)
            nc.gpsimd.memset(ones[:, :], 1.0)
            nc.sync.dma_start(out=pmat[:, :], in_=pmat_d)
            if mode == "causal":
                bm = cpool.tile([128, 4 * TQ], BF16)
                nc.sync.dma_start(out=bm.rearrange("p (r f) -> p r f", r=4),
                                  in_=bmask_d.rearrange("r p f -> p r f"))

            # ---- residents ----
            # qT/kT per instance [128, T] bf16; vres per batch [128, NTK*DL];
            # attnvT per instance [128, T] bf16
            qT = [rpool.tile([128, T], BF16, name=f"qT{i}") for i in range(4)]
            kTr = [rpool.tile([128, T], BF16, name=f"kT{i}") for i in range(4)]
            vres = [rpool.tile([128, NTK * DL], BF16, name=f"vres{b}")
                    for b in range(B)]
            avT = [rpool.tile([128, T], BF16, name=f"avT{i}") for i in range(4)]

            def inst(b, h):
                return b * HPC + h

            xT3 = xT_d.rearrange("(k p) t -> p k t", p=128)

            # =================== Phase A: projections + RoPE ================
            for blk in range(B * NTQ):
                b, tb = blk // NTQ, blk % NTQ
                toff = tb * TQ
                coff = b * T + toff  # column offset in xT
                xt = xpool.tile([128, KT * TQ], BF16, tag="x")
                nc.sync.dma_start(
                    out=xt.rearrange("p (k t) -> p k t", k=KT),
                    in_=xT3[:, :, coff:coff + TQ])

                kf_tiles = []
                for h in range(HPC):
                    # ---- q ----
                    qp = ps_s.tile([128, TQ], F32, tag="s")
                    for k in range(KT):
                        nc.tensor.matmul(
                            qp[:, :], wqt[:, k * DL + h * HD:k * DL + (h + 1) * HD],
                            xt[:, k * TQ:(k + 1) * TQ],
                            start=(k == 0), stop=(k == KT - 1))
                    t1 = tpool.tile([128, TQ], F32, tag="t1")
                    t2 = tpool.tile([128, TQ], F32, tag="t2")
                    nc.vector.tensor_tensor(out=t1[:, :], in0=qp[:, :],
                                            in1=cost[:, toff:toff + TQ],
                                            op=mybir.AluOpType.mult)
                    # t2 = swap(A) * sint  (sint rows: [-sin; +sin]); psum
                    # input may have a different base partition than sbuf.
                    nc.vector.tensor_tensor(out=t2[0:64, :], in0=qp[64:128, :],
                                            in1=sint[0:64, toff:toff + TQ],
                                            op=mybir.AluOpType.mult)
                    nc.vector.tensor_tensor(out=t2[64:128, :], in0=qp[0:64, :],
                                            in1=sint[64:128, toff:toff + TQ],
                                            op=mybir.AluOpType.mult)
                    q_dst = qT[inst(b, h)]
                    nc.vector.tensor_tensor(out=q_dst[:, toff:toff + TQ],
                                            in0=t1[:, :], in1=t2[:, :],
                                            op=mybir.AluOpType.add)
                    # ---- k ----
                    kp = ps_s.tile([128, TQ], F32, tag="s")
                    for k in range(KT):
                        nc.tensor.matmul(
                            kp[:, :], wkt[:, k * DL + h * HD:k * DL + (h + 1) * HD],
                            xt[:, k * TQ:(k + 1) * TQ],
                            start=(k == 0), stop=(k == KT - 1))
                    t1 = tpool.tile([128, TQ], F32, tag="t1")
                    t2 = tpool.tile([128, TQ], F32, tag="t2")
                    kf = kfpool.tile([128, TQ], F32, tag="kf")
                    nc.vector.tensor_tensor(out=t1[:, :], in0=kp[:, :],
                                            in1=cost[:, toff:toff + TQ],
                                            op=mybir.AluOpType.mult)
                    nc.vector.tensor_tensor(out=t2[0:64, :], in0=kp[64:128, :],
                                            in1=sint[0:64, toff:toff + TQ],
                                            op=mybir.AluOpType.mult)
                    nc.vector.tensor_tensor(out=t2[64:128, :], in0=kp[0:64, :],
                                            in1=sint[64:128, toff:toff + TQ],
                                            op=mybir.AluOpType.mult)
                    nc.vector.tensor_tensor(out=kf[:, :],
                                            in0=t1[:, :], in1=t2[:, :],
                                            op=mybir.AluOpType.add)
                    nc.vector.tensor_copy(kTr[inst(b, h)][:, toff:toff + TQ],
                                          kf[:, :])
                    kf_tiles.append(kf)
                # k-cache: transpose roped-k blocks to [t, d] (both heads
                # into one psum tile, then one 1KB-burst DMA per t-row block)
                for sub in range(4):
                    ktp = ps_sm.tile([128, DL], F32, tag="sm")
                    for h in range(HPC):
                        nc.tensor.transpose(ktp[:, h * HD:(h + 1) * HD],
                                            kf_tiles[h][:, sub * 128:(sub + 1) * 128],
                                            pmat[:, :])
                    ks = kvspool.tile([128, DL], F32, tag="ks")
                    nc.scalar.copy(ks[:, :], ktp[:, :])
                    trow = toff + sub * 128
                    nc.sync.dma_start(out=k_d[b, trow:trow + 128, :, :],
                                      in_=ks[:, :])
                # ---- v (normal layout: lhsT = x tile, rhs = wvT) ----
                for sub in range(4):
                    vp = ps_sm.tile([128, DL], F32, tag="sm")
                    for k in range(KT):
                        nc.tensor.matmul(
                            vp[:, :], xt[:, k * TQ + sub * 128:k * TQ + (sub + 1) * 128],
                            wvt[:, k * DL:(k + 1) * DL],
                            start=(k == 0), stop=(k == KT - 1))
                    vs = kvspool.tile([128, DL], F32, tag="vs")
                    nc.scalar.copy(vs[:, :], vp[:, :])
                    trow = toff + sub * 128
                    nc.sync.dma_start(out=v_d[b, trow:trow + 128, :, :],
                                      in_=vs[:, :])
                    ig = tb * 4 + sub
                    nc.gpsimd.tensor_copy(vres[b][:, ig * DL:(ig + 1) * DL],
                                          vs[:, :])

            # =================== Phase B: attention =========================
            # Interleave the two head-instances of each batch so the PE
            # always has an independent QK/PV chain while one waits on exp.
            def attn_block(b, h, j):
                i_ = inst(b, h)
                ntk = (4 * j + 4) if mode == "causal" else NTK
                pv = ps_pv.tile([128, TQ], F32, tag="pv", name=f"pv{i_}_{j}")
                dn = ps_sm.tile([1, TQ], F32, tag="sm", name=f"dn{i_}_{j}")
                for i in range(ntk):
                    # diagonal blocks: columns tq < 128r are fully masked
                    r = i - 4 * j
                    n0 = 128 * r if (mode == "causal" and r > 0) else 0
                    sl = slice(n0, TQ)
                    sp = ps_s.tile([128, TQ], F32, tag="s", name=f"sp{i_}_{j}_{i}")
                    nc.tensor.matmul(sp[:, sl],
                                     kTr[i_][:, i * TK:(i + 1) * TK],
                                     qT[i_][:, j * TQ + n0:(j + 1) * TQ],
                                     start=True, stop=True)
                    if mode == "general":
                        mk = mkpool.tile([128, TQ], F32, tag="mk", name=f"mk{i_}_{j}_{i}")
                        nc.sync.dma_start(
                            out=mk[:, :],
                            in_=maskTs_d[i * TK:(i + 1) * TK,
                                         j * TQ:(j + 1) * TQ])
                        nc.vector.tensor_tensor(
                            out=sp[:, :], in0=sp[:, :],
                            in1=mk[:, :],
                            op=mybir.AluOpType.add)
                    et = etpool.tile([128, TQ], BF16, tag="et", name=f"et{i_}_{j}_{i}")
                    nc.scalar.activation(et[:, sl], sp[:, sl],
                                         mybir.ActivationFunctionType.Exp,
                                         bias=0.0, scale=SCALE)
                    if mode == "causal" and r >= 0:
                        nc.vector.tensor_tensor(
                            out=et[:, sl], in0=et[:, sl],
                            in1=bm[:, r * TQ + n0:(r + 1) * TQ],
                            op=mybir.AluOpType.mult)
                    nc.tensor.matmul(pv[:, sl],
                                     vres[b][:, i * DL + h * HD:i * DL + (h + 1) * HD],
                                     et[:, sl],
                                     start=(i == 0), stop=(i == ntk - 1))
                    nc.tensor.matmul(dn[:, sl], ones[:, :], et[:, sl],
                                     start=(i == 0), stop=(i == ntk - 1))
                rc = rowpool.tile([1, TQ], F32, tag="rc", name=f"rc{i_}_{j}")
                nc.vector.reciprocal_approx_fast(out=rc[:, :], in_=dn[:1, :])
                bc = bcpool.tile([128, TQ], F32, tag="bc", name=f"bc{i_}_{j}")
                nc.gpsimd.partition_broadcast(bc[:, :], rc[:, :], 128)
                nc.vector.tensor_tensor(out=avT[i_][:, j * TQ:(j + 1) * TQ],
                                        in0=pv[:, :], in1=bc[:, :],
                                        op=mybir.AluOpType.mult)

            for b in range(B):
                for j in range(NTQ):
                    for h in range(HPC):
                        attn_block(b, h, j)

            # =================== Phase C: output projection =================
            for b in range(B):
                for tb in range(T // 128):
                    ost = ostpool.tile([128, D], BF16, tag="ost")
                    for jb in range(NTQ):
                        op = ps_s.tile([128, TQ], F32, tag="s")
                        for dt_ in range(HPC):
                            nc.tensor.matmul(
                                op[:, :],
                                avT[inst(b, dt_)][:, tb * 128:(tb + 1) * 128],
                                wot[:, dt_ * D + jb * TQ:dt_ * D + (jb + 1) * TQ],
                                start=(dt_ == 0), stop=(dt_ == HPC - 1))
                        dst = ost[:, jb * TQ:(jb + 1) * TQ]
                        if jb % 2 == 0:
                            nc.vector.tensor_copy(dst, op[:, :])
                        else:
                            nc.scalar.copy(dst, op[:, :])
                    row = b * T + tb * 128
                    nc.sync.dma_start(out=out_d[row:row + 128, :], in_=ost[:, :])

    nc.compile()
    return nc


def _prep_inputs(x, cos, sin, mask, wq, wk, wv, wo, mode):
    bf = ml_dtypes.bfloat16
    xT = np.ascontiguousarray(x.reshape(BT, D).T).astype(bf)
    perm = np.concatenate([np.arange(0, HD, 2), np.arange(1, HD, 2)])
    cosT64 = np.ascontiguousarray(cos.T.astype(np.float32))
    sinT64 = np.ascontiguousarray(sin.T.astype(np.float32))
    cosT = np.vstack([cosT64, cosT64])
    sinT2 = np.vstack([-sinT64, sinT64])
    perm_mat = np.zeros((128, 128), dtype=np.float32)
    perm_mat[np.arange(128), perm] = 1.0
    common = {"xT": xT, "cosT": cosT, "sinT2": sinT2, "pmat": perm_mat}
    if mode == "causal":
        f = np.arange(TQ)[None, :]
        p = np.arange(128)[:, None]
        bmask = np.stack([(128 * r + p <= f) for r in range(4)]).astype(bf)
        common["bmask"] = bmask
    if mode == "general":
        common["maskTs"] = np.ascontiguousarray(
            mask.T.astype(np.float64) / SCALE).astype(np.float32)

    in_maps = []
    for c in range(NCORES):
        rows = []
        for h in range(HPC):
            rows.extend(range((c * HPC + h) * HD, (c * HPC + h) * HD + HD))
        rows = np.array(rows)
        prows = np.concatenate([(c * HPC + h) * HD + perm for h in range(HPC)])
        m = dict(common)
        m["wqT"] = np.ascontiguousarray(wq[prows, :].T).astype(bf)
        m["wkT"] = np.ascontiguousarray(wk[prows, :].T).astype(bf)
        m["wvT"] = np.ascontiguousarray(wv[rows, :].T).astype(bf)
        m["woT"] = np.ascontiguousarray(wo[:, rows].T).astype(bf)
        in_maps.append(m)
    return in_maps


def kernel(x, cos, sin, mask, wq, wk, wv, wo):
    global LAST_RESULTS
    x = np.asarray(x, dtype=np.float32)
    cos = np.asarray(cos, dtype=np.float32)
    sin = np.asarray(sin, dtype=np.float32)
    mask = np.asarray(mask, dtype=np.float32)
    wq = np.asarray(wq, dtype=np.float32)
    wk = np.asarray(wk, dtype=np.float32)
    wv = np.asarray(wv, dtype=np.float32)
    wo = np.asarray(wo, dtype=np.float32)

    if not mask.any():
        mode = "zeros"
    else:
        tril = np.tril(np.ones((T, T), dtype=bool))
        if np.array_equal(mask, np.where(tril, 0.0, -1e9).astype(np.float32)):
            mode = "causal"
        else:
            mode = "general"

    if mode not in _CACHE:
        _CACHE[mode] = _build_program(mode)
    nc = _CACHE[mode]

    in_maps = _prep_inputs(x, cos, sin, mask, wq, wk, wv, wo, mode)
    res = run_bass_kernel_spmd(nc, in_maps, core_ids=list(range(NCORES)))
    LAST_RESULTS = res

    out = np.zeros((BT, D), dtype=np.float32)
    k_full = np.empty((B, T, H, HD), dtype=np.float32)
    v_full = np.empty((B, T, H, HD), dtype=np.float32)
    for c in range(NCORES):
        r = res.results[c]
        out += r["part"].astype(np.float32)
        k_full[:, :, c * HPC:(c + 1) * HPC, :] = r["kc"]
        v_full[:, :, c * HPC:(c + 1) * HPC, :] = r["vc"]
    return out.reshape(B, T, D), k_full, v_full


# revision 22
# speedup vs baseline: 1.0131x; 1.0131x over previous
"""Causal self-attention (B=2, T=2048, D=2048, H=16, HD=128) on 8 trn2 cores.

Sharding: tensor-parallel over heads (2 heads/core). Each core computes
q/k/v projections for its heads from the full input, runs attention, and
produces a partial output projection (its heads' slice of wo's contraction);
the host sums the 8 partials. k/v caches are written directly per head-slice.

Layout choices:
  - x is pre-transposed on host to xT [D, B*T] so projections produce
    qT/kT [HD, T] (head dim on partitions) with no on-chip transposes.
  - q/k head-dims are permuted (even indices first, odd second) via a host
    permutation of the wq/wk rows, so RoPE's even/odd pair-mixing becomes
    partition-half arithmetic on the DVE. The permutation cancels in q.kT.
  - scores are computed transposed (scoresT [tk, tq]) so the exp'd
    probabilities feed the PV matmul directly as the moving operand.
    Softmax is max-free (inputs are well-scaled); the denominator comes from
    an ones-vector matmul accumulated alongside PV, and is applied to
    attnvT columns via reciprocal + gpsimd partition-broadcast + DVE mult.
  - The attention output attnvT [d, tq] is exactly the lhsT the wo
    projection needs; its output goes PSUM -> DRAM directly.
"""

import math

import ml_dtypes
import numpy as np

import concourse.bacc as bacc
import concourse.mybir as mybir
import concourse.tile as tile
from concourse.bass_utils import run_bass_kernel_spmd

B, T, D, H, HD = 2, 2048, 2048, 16, 128
NCORES = 8
HPC = H // NCORES          # heads per core
DL = HPC * HD              # local head-dim span (256)
BT = B * T
KT = D // 128              # contraction tiles for projections
TQ = 512                   # query block (free dim of scoresT)
NTQ = T // TQ
TK = 128                   # key block (partition dim of scoresT)
NTK = T // TK
SCALE = 1.0 / math.sqrt(HD)
BF16 = mybir.dt.bfloat16
F32 = mybir.dt.float32

_CACHE = {}
LAST_RESULTS = None


def _build_program(mode):
    """mode: 'causal' (skip masked blocks, binary-mask diagonal),
    'zeros' (no mask at all), 'general' (additive mask streamed from DRAM)."""
    nc = bacc.Bacc("TRN2", target_bir_lowering=False, debug=False,
                   num_devices=NCORES)

    xT_d = nc.dram_tensor("xT", (D, BT), BF16, kind="ExternalInput").ap()
    wqT_d = nc.dram_tensor("wqT", (D, DL), BF16, kind="ExternalInput").ap()
    wkT_d = nc.dram_tensor("wkT", (D, DL), BF16, kind="ExternalInput").ap()
    wvT_d = nc.dram_tensor("wvT", (D, DL), BF16, kind="ExternalInput").ap()
    woT_d = nc.dram_tensor("woT", (DL, D), BF16, kind="ExternalInput").ap()
    cosT_d = nc.dram_tensor("cosT", (128, T), F32, kind="ExternalInput").ap()
    pmat_d = nc.dram_tensor("pmat", (128, 128), F32, kind="ExternalInput").ap()
    sinT2_d = nc.dram_tensor("sinT2", (128, T), F32, kind="ExternalInput").ap()
    if mode == "causal":
        bmask_d = nc.dram_tensor("bmask", (4, 128, TQ), BF16,
                                 kind="ExternalInput").ap()
    if mode == "general":
        maskTs_d = nc.dram_tensor("maskTs", (T, T), F32,
                                  kind="ExternalInput").ap()

    out_d = nc.dram_tensor("part", (BT, D), BF16, kind="ExternalOutput").ap()
    k_d = nc.dram_tensor("kc", (B, T, HPC, HD), F32, kind="ExternalOutput").ap()
    v_d = nc.dram_tensor("vc", (B, T, HPC, HD), F32, kind="ExternalOutput").ap()

    with tile.TileContext(nc) as tc:
        with (
            tc.tile_pool(name="const", bufs=1) as cpool,
            tc.tile_pool(name="resid", bufs=1) as rpool,
            tc.tile_pool(name="xt", bufs=2) as xpool,
            tc.tile_pool(name="rope", bufs=4) as tpool,
            tc.tile_pool(name="kf", bufs=3) as kfpool,
            tc.tile_pool(name="mk", bufs=3) as mkpool,
            tc.tile_pool(name="et", bufs=6) as etpool,
            tc.tile_pool(name="row", bufs=2) as rowpool,
            tc.tile_pool(name="bcast", bufs=2) as bcpool,
            tc.tile_pool(name="kvs", bufs=3) as kvspool,
            tc.tile_pool(name="os", bufs=3) as ospool,
            tc.tile_pool(name="ost", bufs=2) as ostpool,
            tc.tile_pool(name="ps_s", bufs=4, space="PSUM") as ps_s,
            tc.tile_pool(name="ps_sm", bufs=2, space="PSUM") as ps_sm,
            tc.tile_pool(name="ps_pv", bufs=2, space="PSUM") as ps_pv,
        ):
            # ---- constants ----
            wqt = cpool.tile([128, KT * DL], BF16)
            wkt = cpool.tile([128, KT * DL], BF16)
            wvt = cpool.tile([128, KT * DL], BF16)
            wot = cpool.tile([128, 2 * D], BF16)
            cost = cpool.tile([128, T], F32)
            sint = cpool.tile([128, T], F32)
            ones = cpool.tile([128, 1], BF16)
            pmat = cpool.tile([128, 128], F32)
            nc.sync.dma_start(out=wqt.rearrange("p (k m) -> p k m", k=KT),
                              in_=wqT_d.rearrange("(k p) m -> p k m", p=128))
            nc.sync.dma_start(out=wkt.rearrange("p (k m) -> p k m", k=KT),
                              in_=wkT_d.rearrange("(k p) m -> p k m", p=128))
            nc.sync.dma_start(out=wvt.rearrange("p (k m) -> p k m", k=KT),
                              in_=wvT_d.rearrange("(k p) m -> p k m", p=128))
            nc.sync.dma_start(out=wot.rearrange("p (k m) -> p k m", k=2),
                              in_=woT_d.rearrange("(k p) m -> p k m", p=128))
            nc.sync.dma_start(out=cost[:, :], in_=cosT_d)
            nc.sync.dma_start(out=sint[:, :], in_=sinT2_d)
            nc.gpsimd.memset(ones[:, :], 1.0)
            nc.sync.dma_start(out=pmat[:, :], in_=pmat_d)
            if mode == "causal":
                bm = cpool.tile([128, 4 * TQ], BF16)
                nc.sync.dma_start(out=bm.rearrange("p (r f) -> p r f", r=4),
                                  in_=bmask_d.rearrange("r p f -> p r f"))

            # ---- residents ----
            # qT/kT per instance [128, T] bf16; vres per batch [128, NTK*DL];
            # attnvT per instance [128, T] bf16
            qT = [rpool.tile([128, T], BF16, name=f"qT{i}") for i in range(4)]
            kTr = [rpool.tile([128, T], BF16, name=f"kT{i}") for i in range(4)]
            vres = [rpool.tile([128, NTK * DL], BF16, name=f"vres{b}")
                    for b in range(B)]
            avT = [rpool.tile([128, T], BF16, name=f"avT{i}") for i in range(4)]

            def inst(b, h):
                return b * HPC + h

            xT3 = xT_d.rearrange("(k p) t -> p k t", p=128)

            # ============ Phase A (per batch): projections + RoPE ==========
            def phase_a(b):
              for tb in range(NTQ):
                toff = tb * TQ
                coff = b * T + toff  # column offset in xT
                xt = xpool.tile([128, KT * TQ], BF16, tag="x")
                nc.sync.dma_start(
                    out=xt.rearrange("p (k t) -> p k t", k=KT),
                    in_=xT3[:, :, coff:coff + TQ])

                kf_tiles = []
                for h in range(HPC):
                    # ---- q ----
                    qp = ps_s.tile([128, TQ], F32, tag="s")
                    for k in range(KT):
                        nc.tensor.matmul(
                            qp[:, :], wqt[:, k * DL + h * HD:k * DL + (h + 1) * HD],
                            xt[:, k * TQ:(k + 1) * TQ],
                            start=(k == 0), stop=(k == KT - 1))
                    t1 = tpool.tile([128, TQ], F32, tag="t1")
                    t2 = tpool.tile([128, TQ], F32, tag="t2")
                    nc.vector.tensor_tensor(out=t1[:, :], in0=qp[:, :],
                                            in1=cost[:, toff:toff + TQ],
                                            op=mybir.AluOpType.mult)
                    # t2 = swap(A) * sint  (sint rows: [-sin; +sin]); psum
                    # input may have a different base partition than sbuf.
                    nc.vector.tensor_tensor(out=t2[0:64, :], in0=qp[64:128, :],
                                            in1=sint[0:64, toff:toff + TQ],
                                            op=mybir.AluOpType.mult)
                    nc.vector.tensor_tensor(out=t2[64:128, :], in0=qp[0:64, :],
                                            in1=sint[64:128, toff:toff + TQ],
                                            op=mybir.AluOpType.mult)
                    q_dst = qT[inst(b, h)]
                    nc.vector.tensor_tensor(out=q_dst[:, toff:toff + TQ],
                                            in0=t1[:, :], in1=t2[:, :],
                                            op=mybir.AluOpType.add)
                    # ---- k ----
                    kp = ps_s.tile([128, TQ], F32, tag="s")
                    for k in range(KT):
                        nc.tensor.matmul(
                            kp[:, :], wkt[:, k * DL + h * HD:k * DL + (h + 1) * HD],
                            xt[:, k * TQ:(k + 1) * TQ],
                            start=(k == 0), stop=(k == KT - 1))
                    t1 = tpool.tile([128, TQ], F32, tag="t1")
                    t2 = tpool.tile([128, TQ], F32, tag="t2")
                    kf = kfpool.tile([128, TQ], F32, tag="kf")
                    nc.vector.tensor_tensor(out=t1[:, :], in0=kp[:, :],
                                            in1=cost[:, toff:toff + TQ],
                                            op=mybir.AluOpType.mult)
                    nc.vector.tensor_tensor(out=t2[0:64, :], in0=kp[64:128, :],
                                            in1=sint[0:64, toff:toff + TQ],
                                            op=mybir.AluOpType.mult)
                    nc.vector.tensor_tensor(out=t2[64:128, :], in0=kp[0:64, :],
                                            in1=sint[64:128, toff:toff + TQ],
                                            op=mybir.AluOpType.mult)
                    nc.vector.tensor_tensor(out=kf[:, :],
                                            in0=t1[:, :], in1=t2[:, :],
                                            op=mybir.AluOpType.add)
                    nc.vector.tensor_copy(kTr[inst(b, h)][:, toff:toff + TQ],
                                          kf[:, :])
                    kf_tiles.append(kf)
                # k-cache: transpose roped-k blocks to [t, d] (both heads
                # into one psum tile, then one 1KB-burst DMA per t-row block)
                for sub in range(4):
                    ktp = ps_sm.tile([128, DL], F32, tag="sm")
                    for h in range(HPC):
                        nc.tensor.transpose(ktp[:, h * HD:(h + 1) * HD],
                                            kf_tiles[h][:, sub * 128:(sub + 1) * 128],
                                            pmat[:, :])
                    ks = kvspool.tile([128, DL], F32, tag="ks")
                    nc.scalar.copy(ks[:, :], ktp[:, :])
                    trow = toff + sub * 128
                    nc.sync.dma_start(out=k_d[b, trow:trow + 128, :, :],
                                      in_=ks[:, :])
                # ---- v (normal layout: lhsT = x tile, rhs = wvT) ----
                for sub in range(4):
                    vp = ps_sm.tile([128, DL], F32, tag="sm")
                    for k in range(KT):
                        nc.tensor.matmul(
                            vp[:, :], xt[:, k * TQ + sub * 128:k * TQ + (sub + 1) * 128],
                            wvt[:, k * DL:(k + 1) * DL],
                            start=(k == 0), stop=(k == KT - 1))
                    vs = kvspool.tile([128, DL], F32, tag="vs")
                    nc.scalar.copy(vs[:, :], vp[:, :])
                    trow = toff + sub * 128
                    nc.sync.dma_start(out=v_d[b, trow:trow + 128, :, :],
                                      in_=vs[:, :])
                    ig = tb * 4 + sub
                    nc.gpsimd.tensor_copy(vres[b][:, ig * DL:(ig + 1) * DL],
                                          vs[:, :])

            # =================== Phase B: attention =========================
            # Interleave the two head-instances of each batch so the PE
            # always has an independent QK/PV chain while one waits on exp.
            def attn_block(b, h, j):
                i_ = inst(b, h)
                ntk = (4 * j + 4) if mode == "causal" else NTK
                pv = ps_pv.tile([128, TQ], F32, tag="pv", name=f"pv{i_}_{j}")
                dn = ps_sm.tile([1, TQ], F32, tag="sm", name=f"dn{i_}_{j}")
                for i in range(ntk):
                    # diagonal blocks: columns tq < 128r are fully masked
                    r = i - 4 * j
                    n0 = 128 * r if (mode == "causal" and r > 0) else 0
                    sl = slice(n0, TQ)
                    sp = ps_s.tile([128, TQ], F32, tag="s", name=f"sp{i_}_{j}_{i}")
                    nc.tensor.matmul(sp[:, sl],
                                     kTr[i_][:, i * TK:(i + 1) * TK],
                                     qT[i_][:, j * TQ + n0:(j + 1) * TQ],
                                     start=True, stop=True)
                    if mode == "general":
                        mk = mkpool.tile([128, TQ], F32, tag="mk", name=f"mk{i_}_{j}_{i}")
                        nc.sync.dma_start(
                            out=mk[:, :],
                            in_=maskTs_d[i * TK:(i + 1) * TK,
                                         j * TQ:(j + 1) * TQ])
                        nc.vector.tensor_tensor(
                            out=sp[:, :], in0=sp[:, :],
                            in1=mk[:, :],
                            op=mybir.AluOpType.add)
                    et = etpool.tile([128, TQ], BF16, tag="et", name=f"et{i_}_{j}_{i}")
                    nc.scalar.activation(et[:, sl], sp[:, sl],
                                         mybir.ActivationFunctionType.Exp,
                                         bias=0.0, scale=SCALE)
                    if mode == "causal" and r >= 0:
                        nc.vector.tensor_tensor(
                            out=et[:, sl], in0=et[:, sl],
                            in1=bm[:, r * TQ + n0:(r + 1) * TQ],
                            op=mybir.AluOpType.mult)
                    nc.tensor.matmul(pv[:, sl],
                                     vres[b][:, i * DL + h * HD:i * DL + (h + 1) * HD],
                                     et[:, sl],
                                     start=(i == 0), stop=(i == ntk - 1))
                    nc.tensor.matmul(dn[:, sl], ones[:, :], et[:, sl],
                                     start=(i == 0), stop=(i == ntk - 1))
                rc = rowpool.tile([1, TQ], F32, tag="rc", name=f"rc{i_}_{j}")
                nc.vector.reciprocal_approx_fast(out=rc[:, :], in_=dn[:1, :])
                bc = bcpool.tile([128, TQ], F32, tag="bc", name=f"bc{i_}_{j}")
                nc.gpsimd.partition_broadcast(bc[:, :], rc[:, :], 128)
                nc.vector.tensor_tensor(out=avT[i_][:, j * TQ:(j + 1) * TQ],
                                        in0=pv[:, :], in1=bc[:, :],
                                        op=mybir.AluOpType.mult)

            def phase_b(b):
                for j in range(NTQ):
                    for h in range(HPC):
                        attn_block(b, h, j)

            # =================== Phase C: output projection =================
            def phase_c(b):
                for tb in range(T // 128):
                    ost = ostpool.tile([128, D], BF16, tag="ost")
                    for jb in range(NTQ):
                        op = ps_s.tile([128, TQ], F32, tag="s")
                        for dt_ in range(HPC):
                            nc.tensor.matmul(
                                op[:, :],
                                avT[inst(b, dt_)][:, tb * 128:(tb + 1) * 128],
                                wot[:, dt_ * D + jb * TQ:dt_ * D + (jb + 1) * TQ],
                                start=(dt_ == 0), stop=(dt_ == HPC - 1))
                        dst = ost[:, jb * TQ:(jb + 1) * TQ]
                        if jb % 2 == 0:
                            nc.vector.tensor_copy(dst, op[:, :])
                        else:
                            nc.scalar.copy(dst, op[:, :])
                    row = b * T + tb * 128
                    nc.sync.dma_start(out=out_d[row:row + 128, :], in_=ost[:, :])

            for b in range(B):
                phase_a(b)
                phase_b(b)
                phase_c(b)

    nc.compile()
    return nc


def _prep_inputs(x, cos, sin, mask, wq, wk, wv, wo, mode):
    bf = ml_dtypes.bfloat16
    xT = np.ascontiguousarray(x.reshape(BT, D).T).astype(bf)
    perm = np.concatenate([np.arange(0, HD, 2), np.arange(1, HD, 2)])
    cosT64 = np.ascontiguousarray(cos.T.astype(np.float32))
    sinT64 = np.ascontiguousarray(sin.T.astype(np.float32))
    cosT = np.vstack([cosT64, cosT64])
    sinT2 = np.vstack([-sinT64, sinT64])
    perm_mat = np.zeros((128, 128), dtype=np.float32)
    perm_mat[np.arange(128), perm] = 1.0
    common = {"xT": xT, "cosT": cosT, "sinT2": sinT2, "pmat": perm_mat}
    if mode == "causal":
        f = np.arange(TQ)[None, :]
        p = np.arange(128)[:, None]
        bmask = np.stack([(128 * r + p <= f) for r in range(4)]).astype(bf)
        common["bmask"] = bmask
    if mode == "general":
        common["maskTs"] = np.ascontiguousarray(
            mask.T.astype(np.float64) / SCALE).astype(np.float32)

    in_maps = []
    for c in range(NCORES):
        rows = []
        for h in range(HPC):
            rows.extend(range((c * HPC + h) * HD, (c * HPC + h) * HD + HD))
        rows = np.array(rows)
        prows = np.concatenate([(c * HPC + h) * HD + perm for h in range(HPC)])
        m = dict(common)
        m["wqT"] = np.ascontiguousarray(wq[prows, :].T).astype(bf)
        m["wkT"] = np.ascontiguousarray(wk[prows, :].T).astype(bf)
        m["wvT"] = np.ascontiguousarray(wv[rows, :].T).astype(bf)
        m["woT"] = np.ascontiguousarray(wo[:, rows].T).astype(bf)
        in_maps.append(m)
    return in_maps


def kernel(x, cos, sin, mask, wq, wk, wv, wo):
    global LAST_RESULTS
    x = np.asarray(x, dtype=np.float32)
    cos = np.asarray(cos, dtype=np.float32)
    sin = np.asarray(sin, dtype=np.float32)
    mask = np.asarray(mask, dtype=np.float32)
    wq = np.asarray(wq, dtype=np.float32)
    wk = np.asarray(wk, dtype=np.float32)
    wv = np.asarray(wv, dtype=np.float32)
    wo = np.asarray(wo, dtype=np.float32)

    if not mask.any():
        mode = "zeros"
    else:
        tril = np.tril(np.ones((T, T), dtype=bool))
        if np.array_equal(mask, np.where(tril, 0.0, -1e9).astype(np.float32)):
            mode = "causal"
        else:
            mode = "general"

    if mode not in _CACHE:
        _CACHE[mode] = _build_program(mode)
    nc = _CACHE[mode]

    in_maps = _prep_inputs(x, cos, sin, mask, wq, wk, wv, wo, mode)
    res = run_bass_kernel_spmd(nc, in_maps, core_ids=list(range(NCORES)))
    LAST_RESULTS = res

    out = np.zeros((BT, D), dtype=np.float32)
    k_full = np.empty((B, T, H, HD), dtype=np.float32)
    v_full = np.empty((B, T, H, HD), dtype=np.float32)
    for c in range(NCORES):
        r = res.results[c]
        out += r["part"].astype(np.float32)
        k_full[:, :, c * HPC:(c + 1) * HPC, :] = r["kc"]
        v_full[:, :, c * HPC:(c + 1) * HPC, :] = r["vc"]
    return out.reshape(B, T, D), k_full, v_full
